# revision 3
# baseline (speedup 1.0000x reference)
"""BGE-M3 sparse-embedding head (matvec + relu + scatter-max into (B, V))
as a Bass/Tile kernel on 8 Trainium2 NeuronCores.

Sharding: data-parallel over batch; each core computes 4 of 32 rows.

Per core:
  1. tw = relu(hidden @ w + b) streamed in 128-token tiles, computed with a
     fused scalar_tensor_tensor (multiply + free-dim sum) on the vector engine.
  2. Each output row is zero-filled by direct DMAs from a memset SBUF tile,
     then every singleton (non-special, non-duplicate) token's weight is
     written with ONE 128-index indirect DMA per 128-token chunk: the host
     precomputes the scatter target map sgid[p, k] = vocab id (or V = OOB
     skip).  Distinct vocab ids -> distinct addresses, so no ordering hazards
     beyond the row's zero-fill (enforced by tile's DRAM dependency tracking).
  3. Duplicate vocab ids within a row (a handful per row; the class structure
     is a pure function of input_ids, so the host computes it) are excluded
     from the scatter map and resolved exactly by a small bf16 hi/lo matmul
     pair that buckets each class's member weights into one PSUM row, a
     free-dim reduce_max, and ONE 128-index indirect DMA per row (targets
     disjoint from the singleton scatters).
Special tokens 0..3 are never routed, leaving zeros from the zero-fill.
"""

import ml_dtypes
import numpy as np

import concourse.bass as bass
import concourse.mybir as mybir
import concourse.tile as tile
from concourse.bass import IndirectOffsetOnAxis
from concourse.bass_utils import run_bass_kernel_spmd

V = 250002
NCORES = 8
B, L, H = 32, 1024, 1024
BS = B // NCORES            # batch rows per core
NT = BS * L                 # tokens per core
P = 128
CPR = L // P                # chunks per row (8)
NCHUNK = NT // P            # chunks per core (32)
W = 1954                    # zero/out row width per partition (128*1954 >= V)
MAXCLS = P                  # fixup classes per row (<=128)
MAXMEM = 8                  # members per duplicate class
F32 = mybir.dt.float32
BF16 = mybir.dt.bfloat16
I32 = mybir.dt.int32

_MAX_WAITS = 1


def _split_excess_waits(nc, cap=_MAX_WAITS):
    """walrus's gen3 codegen rejects >1 sync-wait per instruction; move the
    excess onto NoOps inserted just before (same engine => order kept)."""
    n = 0
    for func in nc.m.functions:
        for bb in func.blocks:
            newlist = []
            for ins in bb.instructions:
                si = getattr(ins, "sync_info", None)
                if si is not None and si.on_wait and len(si.on_wait) > cap:
                    waits = list(si.on_wait)
                    extra, keep = waits[:-cap], waits[-cap:]
                    while extra:
                        chunk, extra = extra[:cap], extra[cap:]
                        nop = mybir.InstNoOp(
                            name=f"{ins.name}-wsplit-{n}", ins=[], outs=[]
                        )
                        nop.engine = ins.engine
                        nop.sync_info = mybir.SyncInfo(on_wait=chunk, on_update=[])
                        newlist.append(nop)
                        n += 1
                    ins.sync_info = mybir.SyncInfo(
                        on_wait=keep, on_update=list(si.on_update)
                    )
                newlist.append(ins)
            bb.instructions = newlist
    return n


def _build_program():
    nc = bass.Bass()
    Op = mybir.AluOpType

    hidden = nc.declare_dram_parameter("hidden", [NT, H], F32, isOutput=False)
    wrep = nc.declare_dram_parameter("wrep", [P, H], F32, isOutput=False)
    bcol = nc.declare_dram_parameter("bcol", [P, 1], F32, isOutput=False)
    iota_p = nc.declare_dram_parameter("iota_p", [P, P], BF16, isOutput=False)
    clscol = nc.declare_dram_parameter("clscol", [P, NCHUNK], F32, isOutput=False)
    mkall = nc.declare_dram_parameter(
        "mkall", [P, NCHUNK * MAXMEM], BF16, isOutput=False
    )
    fixgid = nc.declare_dram_parameter("fixgid", [P, BS], I32, isOutput=False)
    sgid = nc.declare_dram_parameter("sgid", [P, NCHUNK], I32, isOutput=False)
    outs = [
        nc.declare_dram_parameter(f"out{r}", [V], F32, isOutput=True)
        for r in range(BS)
    ]

    with tile.TileContext(nc) as tc:
        with (
            tc.tile_pool(name="stream", bufs=4) as stream_tp,
            tc.tile_pool(name="junk", bufs=2) as junk_tp,
            tc.tile_pool(name="lk", bufs=4) as lk_tp,
            tc.tile_pool(name="psumf", bufs=2, space="PSUM") as psumf_tp,
            tc.tile_pool(name="persist", bufs=1) as pers_tp,
        ):
            # ---- one-time loads ----
            wt = pers_tp.tile([P, H], F32, tag="wt")
            nc.sync.dma_start(out=wt[:], in_=wrep[:])
            ip = pers_tp.tile([P, P], BF16, tag="ip")
            nc.sync.dma_start(out=ip[:], in_=iota_p[:])
            cls_t = pers_tp.tile([P, NCHUNK], F32, tag="cls")
            nc.sync.dma_start(out=cls_t[:], in_=clscol[:])
            mk_t = pers_tp.tile([P, NCHUNK * MAXMEM], BF16, tag="mk")
            nc.sync.dma_start(out=mk_t[:], in_=mkall[:])
            bcol_t = pers_tp.tile([P, 1], F32, tag="bcol")
            nc.sync.dma_start(out=bcol_t[:], in_=bcol[:])
            fg_t = pers_tp.tile([P, BS], I32, tag="fg")
            nc.sync.dma_start(out=fg_t[:], in_=fixgid[:])
            sg_t = pers_tp.tile([P, NCHUNK], I32, tag="sg")
            nc.sync.dma_start(out=sg_t[:], in_=sgid[:])

            zt = pers_tp.tile([P, W], F32, tag="zt")
            nc.gpsimd.memset(zt[:], 0.0)

            twraw = pers_tp.tile([P, NCHUNK], F32, tag="twraw")
            tw = pers_tp.tile([P, NCHUNK], F32, tag="tw")
            twbf = pers_tp.tile([P, NCHUNK], BF16, tag="twbf")
            twlo = pers_tp.tile([P, NCHUNK], F32, tag="twlo")
            fixv = pers_tp.tile([P, BS], F32, tag="fixv")

            for r in range(BS):
                # ---- zero-fill this row (from the memset tile) ----
                for si, (p0, p1) in enumerate(
                    ((0, 32), (32, 64), (64, 96), (96, 127))
                ):
                    oeng = nc.scalar if si % 2 == 0 else nc.sync
                    oeng.dma_start(
                        out=outs[r][p0 * W : p1 * W].rearrange("(p f) -> p f", f=W),
                        in_=zt[p0:p1, :],
                    )
                nc.sync.dma_start(
                    out=outs[r][127 * W : V].rearrange("(a f) -> a f", a=1),
                    in_=zt[127:128, 0 : V - 127 * W],
                )

                psf = psumf_tp.tile([P, MAXMEM], F32, tag="psf")
                for j in range(CPR):
                    k = r * CPR + j
                    # ---- matvec for this chunk ----
                    x = stream_tp.tile([P, H], F32, tag="x")
                    deng = nc.sync if j % 2 == 0 else nc.scalar
                    deng.dma_start(out=x[:], in_=hidden[k * P : (k + 1) * P, :])
                    junk = junk_tp.tile([P, H], F32, tag="junk")
                    nc.vector.scalar_tensor_tensor(
                        out=junk[:], in0=x[:], scalar=1.0, in1=wt[:],
                        op0=Op.mult, op1=Op.mult,
                        accum_out=twraw[:, k : k + 1],
                    )
                    # bias + relu
                    nc.vector.tensor_scalar(
                        out=tw[:, k : k + 1], in0=twraw[:, k : k + 1],
                        scalar1=bcol_t[:, 0:1], scalar2=0.0,
                        op0=Op.add, op1=Op.max,
                    )
                    # ---- singleton scatter for this chunk ----
                    nc.gpsimd.indirect_dma_start(
                        out=outs[r][:].unsqueeze(1),
                        out_offset=IndirectOffsetOnAxis(
                            ap=sg_t[:, k : k + 1], axis=0
                        ),
                        in_=tw[:, k : k + 1],
                        in_offset=None,
                        bounds_check=V - 1,
                        oob_is_err=False,
                    )
                    # ---- fixup: bucket duplicate-class member weights ----
                    nc.vector.tensor_copy(
                        out=twbf[:, k : k + 1], in_=tw[:, k : k + 1]
                    )
                    nc.vector.tensor_tensor(
                        out=twlo[:, k : k + 1], in0=tw[:, k : k + 1],
                        in1=twbf[:, k : k + 1], op=Op.subtract,
                    )
                    lkh = lk_tp.tile([P, P], BF16, tag="lkh")
                    nc.vector.tensor_scalar(
                        out=lkh[:], in0=ip[:],
                        scalar1=cls_t[:, k : k + 1], scalar2=tw[:, k : k + 1],
                        op0=Op.is_equal, op1=Op.mult,
                    )
                    lkl = lk_tp.tile([P, P], BF16, tag="lkl")
                    nc.vector.tensor_scalar(
                        out=lkl[:], in0=ip[:],
                        scalar1=cls_t[:, k : k + 1], scalar2=twlo[:, k : k + 1],
                        op0=Op.is_equal, op1=Op.mult,
                    )
                    mk = mk_t[:, k * MAXMEM : (k + 1) * MAXMEM]
                    nc.tensor.matmul(
                        out=psf[:], lhsT=lkh[:], rhs=mk,
                        start=(j == 0), stop=False,
                    )
                    nc.tensor.matmul(
                        out=psf[:], lhsT=lkl[:], rhs=mk,
                        start=False, stop=(j == CPR - 1),
                    )
                # class max over member slots -> per-class fixup values
                nc.vector.tensor_reduce(
                    out=fixv[:, r : r + 1], in_=psf[:],
                    axis=mybir.AxisListType.X, op=Op.max,
                )
                # fixup scatter: one 128-index indirect DMA (D=1), OOB-padded
                nc.gpsimd.indirect_dma_start(
                    out=outs[r][:].unsqueeze(1),
                    out_offset=IndirectOffsetOnAxis(ap=fg_t[:, r : r + 1], axis=0),
                    in_=fixv[:, r : r + 1],
                    in_offset=None,
                    bounds_check=V - 1,
                    oob_is_err=False,
                )

    _split_excess_waits(nc)
    return nc


_prog_cache = {}


def _get_program():
    key = "nc"
    if key not in _prog_cache:
        _prog_cache[key] = _build_program()
    return _prog_cache[key]


def _make_in_maps(hidden_state, input_ids, w_sparse, b_sparse):
    hs = np.asarray(hidden_state, dtype=np.float32).reshape(B, L, H)
    ids_all = np.asarray(input_ids).astype(np.int64).reshape(B, L)
    w = np.asarray(w_sparse, dtype=np.float32).reshape(H)
    bval = float(np.asarray(b_sparse, dtype=np.float32).reshape(-1)[0])

    wrep = np.ascontiguousarray(np.broadcast_to(w, (P, H)))
    bcol = np.full((P, 1), bval, dtype=np.float32)
    iota_p = np.broadcast_to(np.arange(P, dtype=np.float32), (P, P)).astype(
        ml_dtypes.bfloat16
    )

    in_maps = []
    for c in range(NCORES):
        ids = ids_all[c * BS : (c + 1) * BS]                 # (BS, L)
        clsc = np.full((P, NCHUNK), -1.0, np.float32)
        mkall = np.zeros((P, NCHUNK * MAXMEM), np.float32)
        fg = np.full((P, BS), V, np.int32)                   # V => out of bounds
        sg = np.full((P, NCHUNK), V, np.int32)
        for r in range(BS):
            row = ids[r]
            vals, counts = np.unique(row, return_counts=True)
            dupset = {int(v) for v, n in zip(vals, counts) if n > 1 and v >= 4}
            dup_list = sorted(dupset)
            assert len(dup_list) <= MAXCLS, f"too many duplicate classes: {len(dup_list)}"
            clsidx = {v: i for i, v in enumerate(dup_list)}
            memcount = {v: 0 for v in dup_list}
            for q, v in enumerate(dup_list):
                fg[q, r] = v
            for l in range(L):
                tid = int(row[l])
                p, j = l % P, l // P
                k = r * CPR + j
                if tid < 4:
                    continue
                if tid in clsidx:
                    clsc[p, k] = clsidx[tid]
                    m = memcount[tid]
                    assert m < MAXMEM, "duplicate class larger than MAXMEM"
                    mkall[p, k * MAXMEM + m] = 1.0
                    memcount[tid] = m + 1
                else:
                    sg[p, k] = tid
        in_maps.append(
            {
                "hidden": np.ascontiguousarray(
                    hs[c * BS : (c + 1) * BS].reshape(NT, H)
                ),
                "wrep": wrep,
                "bcol": bcol,
                "iota_p": iota_p,
                "clscol": clsc,
                "mkall": mkall.astype(ml_dtypes.bfloat16),
                "fixgid": fg,
                "sgid": sg,
            }
        )
    return in_maps


def kernel(hidden_state, input_ids, w_sparse, b_sparse, _trace=False):
    nc = _get_program()
    in_maps = _make_in_maps(hidden_state, input_ids, w_sparse, b_sparse)
    res = run_bass_kernel_spmd(nc, in_maps, list(range(NCORES)), trace=_trace)
    parts = [
        np.stack([np.asarray(res.results[c][f"out{r}"]) for r in range(BS)])
        for c in range(NCORES)
    ]
    full = np.concatenate(parts, axis=0)
    if _trace:
        kernel.last_exec_time_ns = res.exec_time_ns
        kernel.last_results = res
    return full
